# revision 1
# baseline (speedup 1.0000x reference)
"""GRU-D Trainium2 kernel (8-core SPMD, data-parallel over batch).

Model (reference): B=512, T=200, D=128, H=512.
Per-core: 64 batch samples, full T recurrence.

Decomposition
-------------
All h-independent terms are precomputed as large parallel matmuls (phase A):
    delta_x = min(1, exp(-(d*w_gx + b_gx)))                  [elementwise]
    xhat    = m*x + (1-m)*(delta_x*xl + (1-delta_x)*xm)      [elementwise]
    delta_h = min(1, exp(-(Wgh @ d + b_gh)))                 [D->H matmul]
    P_g     = Wgx_g @ xhat + Wgm_g @ m + b_g   for g in z,r,h
(split of W_g [H, 2D+H] into x / h / m column blocks; comb = [x, h, m]).

The serial scan (phase B) then only needs, per step:
    g  = delta_h_t * h
    z|r = sigmoid(P_zr_t + W{z,r}h @ g)
    u  = r * g
    ht = tanh(P_h_t + Whh @ u)
    h  = g + z*(ht - g)

Everything on-device is stored feature-major: tensors [H, B_local] live as
SBUF tiles [128, 4*64] with column index = h_chunk*64 + b.  Weights are
stationary [128,128] lhsT tiles, the moving operand is the state (N=64).

Phase A streams per-step records to a DRAM scratch laid out
    rec[t] = [dh (256) | Pz (256) | Pr (256) | Ph (256)]   (cols, per partition)
which phase B consumes with one DMA per step.

Final projection (H->2) + batch norm run on host over the gathered
h_last (trivial FLOPs, needs cross-core batch statistics anyway).
"""

import sys

for _p in ("/opt/trn_rl_repo",):
    if _p not in sys.path:
        sys.path.insert(0, _p)

import numpy as np

import concourse.bacc as bacc
import concourse.tile as tile
from concourse import mybir

AF = mybir.ActivationFunctionType
F32 = mybir.dt.float32

B, T_FULL, D, H = 512, 200, 128, 512
NCORES = 8
BL = B // NCORES          # 64 samples per core
MC = H // 128             # 4 h-chunks
REC = 4 * 256             # per-step record width (dh | Pz | Pr | Ph)
CHUNK = 512               # phase-A tb columns per chunk (= 8 steps worth)
TPC = CHUNK // BL         # timesteps per phase-A chunk (8)
BN_EPS = 1e-5

_nc_cache = {}


def build(T=T_FULL):
    """Build the single-core Bass program (SPMD: same program on all cores)."""
    assert T % TPC == 0
    TB = T * BL
    nchunk = TB // CHUNK

    nc = bacc.Bacc("TRN2", target_bir_lowering=False, debug=False)

    # --- external inputs (feature-major, host-prepared) ---
    def din(name, shape):
        return nc.dram_tensor(name, shape, F32, kind="ExternalInput")

    x_d = din("x", [128, TB])
    xl_d = din("xl", [128, TB])
    m_d = din("m", [128, TB])
    dt_d = din("dt", [128, TB])
    xm_d = din("xm", [128, TB])

    wgx_d = din("wgx_n", [128, 1])      # -w_gx
    bgx_d = din("bgx_n", [128, 1])      # -b_gx
    wgh_d = din("wgh_t", [128, H])      # Wgh.T
    bgh_d = din("bgh_n", [128, MC])     # -b_gh  (col = h chunk)

    wxs_d = din("wx_t", [128, 3 * H])   # [Wzx.T | Wrx.T | Whx.T]
    wms_d = din("wm_t", [128, 3 * H])   # [Wzm.T | Wrm.T | Whm.T]
    whh_d = din("wh_t", [128, 3 * MC * H])  # z|r|h hidden blocks, tile (k,m) at
    #                                         g*4096 + k*512 + m*128
    bia_d = din("bias", [128, 3 * MC])  # b_z | b_r | b_h  (col = g*4 + chunk)

    h_out = nc.dram_tensor("h_out", [128, MC * BL], F32, kind="ExternalOutput")

    rec_s = nc.dram_tensor("rec_s", [128, T * REC], F32)

    with tile.TileContext(nc) as tc:
        with (
            tc.tile_pool(name="wsb", bufs=1) as wp,
            tc.tile_pool(name="state", bufs=1) as stp,
        ):
            # resident weights
            wgx = wp.tile([128, 1], F32, tag="wgx")
            bgx = wp.tile([128, 1], F32, tag="bgx")
            wgh = wp.tile([128, H], F32, tag="wgh")
            bgh = wp.tile([128, MC], F32, tag="bgh")
            wxs = wp.tile([128, 3 * H], F32, tag="wxs")
            wms = wp.tile([128, 3 * H], F32, tag="wms")
            whh = wp.tile([128, 3 * MC * H], F32, tag="whh")
            bia = wp.tile([128, 3 * MC], F32, tag="bia")
            for sb_t, dr in [
                (wgx, wgx_d), (bgx, bgx_d), (wgh, wgh_d), (bgh, bgh_d),
                (wxs, wxs_d), (wms, wms_d), (whh, whh_d), (bia, bia_d),
            ]:
                nc.sync.dma_start(sb_t[:], dr[:])

            h = stp.tile([128, MC * BL], F32, tag="h")
            nc.vector.memset(h[:], 0.0)

            # ---------------- phase A ----------------
            with (
                tc.tile_pool(name="pin", bufs=3) as pin,
                tc.tile_pool(name="pw", bufs=2) as pw,
                tc.tile_pool(name="pout", bufs=2) as pout,
                tc.tile_pool(name="psA", bufs=4, space="PSUM") as psA,
            ):
                for ci in range(nchunk):
                    s = ci * CHUNK
                    t0 = ci * TPC
                    xt = pin.tile([128, CHUNK], F32, tag="x")
                    xlt = pin.tile([128, CHUNK], F32, tag="xl")
                    mt = pin.tile([128, CHUNK], F32, tag="m")
                    dtt = pin.tile([128, CHUNK], F32, tag="d")
                    xmt = pin.tile([128, CHUNK], F32, tag="xm")
                    nc.sync.dma_start(xt[:], x_d[:, s:s + CHUNK])
                    nc.sync.dma_start(xlt[:], xl_d[:, s:s + CHUNK])
                    nc.sync.dma_start(mt[:], m_d[:, s:s + CHUNK])
                    nc.sync.dma_start(dtt[:], dt_d[:, s:s + CHUNK])
                    nc.sync.dma_start(xmt[:], xm_d[:, s:s + CHUNK])

                    # delta_h: 4 matmuls [128,128]x[128,512] + exp + min
                    dh_sb = pout.tile([128, TPC, 4 * BL], F32, tag="dh")
                    for mi in range(MC):
                        pdm = psA.tile([128, CHUNK], F32, tag="psA")
                        nc.tensor.matmul(
                            pdm[:], wgh[:, mi * 128:(mi + 1) * 128], dtt[:],
                            start=True, stop=True,
                        )
                        nc.scalar.activation(
                            dh_sb[:, :, mi * BL:(mi + 1) * BL],
                            pdm[:].rearrange("p (t b) -> p t b", b=BL),
                            AF.Exp, bias=bgh[:, mi:mi + 1], scale=-1.0,
                        )
                    dh_flat = dh_sb[:].rearrange("p t b -> p (t b)")
                    nc.vector.tensor_scalar_min(dh_flat, dh_flat, 1.0)
                    rec_v = rec_s[:, t0 * REC:(t0 + TPC) * REC].rearrange(
                        "p (t c) -> p t c", c=REC
                    )
                    nc.sync.dma_start(rec_v[:, :, 0:256], dh_sb[:])

                    # delta_x and xhat (elementwise)
                    dxe = pw.tile([128, CHUNK], F32, tag="dxe")
                    nc.scalar.activation(
                        dxe[:], dtt[:], AF.Exp, bias=bgx[:, 0:1], scale=wgx[:, 0:1]
                    )
                    dx = pw.tile([128, CHUNK], F32, tag="dx")
                    nc.vector.tensor_scalar_min(dx[:], dxe[:], 1.0)
                    t1 = pw.tile([128, CHUNK], F32, tag="t1")
                    nc.vector.tensor_sub(t1[:], xlt[:], xmt[:])
                    t2 = pw.tile([128, CHUNK], F32, tag="t2")
                    nc.vector.tensor_mul(t2[:], dx[:], t1[:])
                    t2b = pw.tile([128, CHUNK], F32, tag="t2b")
                    nc.vector.tensor_add(t2b[:], t2[:], xmt[:])
                    t3 = pw.tile([128, CHUNK], F32, tag="t3")
                    nc.vector.tensor_sub(t3[:], xt[:], t2b[:])
                    t4 = pw.tile([128, CHUNK], F32, tag="t4")
                    nc.vector.tensor_mul(t4[:], mt[:], t3[:])
                    xh = pw.tile([128, CHUNK], F32, tag="xh")
                    nc.vector.tensor_add(xh[:], t4[:], t2b[:])

                    # P_g = Wgx_g @ xhat + Wgm_g @ m + b_g
                    prec = pout.tile([128, TPC, 3 * 4 * BL], F32, tag="prec")
                    for gi in range(3):
                        for mi in range(MC):
                            pp = psA.tile([128, CHUNK], F32, tag="psA")
                            wcol = gi * H + mi * 128
                            nc.tensor.matmul(
                                pp[:], wxs[:, wcol:wcol + 128], xh[:],
                                start=True, stop=False,
                            )
                            nc.tensor.matmul(
                                pp[:], wms[:, wcol:wcol + 128], mt[:],
                                start=False, stop=True,
                            )
                            dst = prec[:, :, gi * 256 + mi * BL:
                                       gi * 256 + (mi + 1) * BL]
                            src = pp[:].rearrange("p (t b) -> p t b", b=BL)
                            b_ap = bia[:, gi * MC + mi:gi * MC + mi + 1]
                            if (gi * MC + mi) % 2 == 0:
                                nc.scalar.activation(
                                    dst, src, AF.Identity, bias=b_ap
                                )
                            else:
                                nc.vector.tensor_scalar_add(dst, src, b_ap)
                    nc.sync.dma_start(rec_v[:, :, 256:1024], prec[:])

            # ---------------- phase B (serial scan) ----------------
            with (
                tc.tile_pool(name="prc", bufs=8) as prc,
                tc.tile_pool(name="pgB", bufs=2) as pgB,
                tc.tile_pool(name="pwB", bufs=2) as pwB,
                tc.tile_pool(name="pzr", bufs=2, space="PSUM") as pzr_p,
                tc.tile_pool(name="pht", bufs=2, space="PSUM") as pht_p,
            ):
                W = MC * BL  # 256
                for t in range(T):
                    rec = prc.tile([128, REC], F32, tag="rec")
                    nc.sync.dma_start(rec[:], rec_s[:, t * REC:(t + 1) * REC])

                    g = pgB.tile([128, W], F32, tag="g")
                    nc.vector.tensor_mul(g[:], rec[:, 0:W], h[:])

                    pzr = pzr_p.tile([128, 512], F32, tag="pzr")
                    for gi in range(2):
                        for mi in range(MC):
                            o = gi * W + mi * BL
                            for k in range(MC):
                                wcol = gi * MC * H + k * H + mi * 128
                                nc.tensor.matmul(
                                    pzr[:, o:o + BL],
                                    whh[:, wcol:wcol + 128],
                                    g[:, k * BL:(k + 1) * BL],
                                    start=(k == 0), stop=(k == MC - 1),
                                )
                    pre = pwB.tile([128, 512], F32, tag="pre")
                    nc.vector.tensor_add(pre[:], pzr[:], rec[:, 256:768])
                    zr = pwB.tile([128, 512], F32, tag="zr")
                    nc.scalar.activation(zr[:], pre[:], AF.Sigmoid)

                    u = pwB.tile([128, W], F32, tag="u")
                    nc.vector.tensor_mul(u[:], zr[:, W:2 * W], g[:])

                    pht = pht_p.tile([128, W], F32, tag="pht")
                    for mi in range(MC):
                        o = mi * BL
                        for k in range(MC):
                            wcol = 2 * MC * H + k * H + mi * 128
                            nc.tensor.matmul(
                                pht[:, o:o + BL],
                                whh[:, wcol:wcol + 128],
                                u[:, k * BL:(k + 1) * BL],
                                start=(k == 0), stop=(k == MC - 1),
                            )
                    preh = pwB.tile([128, W], F32, tag="preh")
                    nc.vector.tensor_add(preh[:], pht[:], rec[:, 768:1024])
                    hts = pwB.tile([128, W], F32, tag="hts")
                    nc.scalar.activation(hts[:], preh[:], AF.Tanh)

                    d1 = pwB.tile([128, W], F32, tag="d1")
                    nc.vector.tensor_sub(d1[:], hts[:], g[:])
                    d2 = pwB.tile([128, W], F32, tag="d2")
                    nc.vector.tensor_mul(d2[:], zr[:, 0:W], d1[:])
                    nc.vector.tensor_add(h[:], g[:], d2[:])

            nc.sync.dma_start(h_out[:], h[:])

    nc.compile()
    return nc


def get_nc(T=T_FULL):
    if T not in _nc_cache:
        _nc_cache[T] = build(T)
    return _nc_cache[T]


# ---------------------------------------------------------------- host prep

def _feature_major(a, Tn):
    """[BL, T, D] -> [D, T*BL] with b fastest."""
    return np.ascontiguousarray(a.transpose(2, 1, 0)).reshape(D, Tn * BL)


def prep_shared(W_gh, b_gh, W_z, b_z, W_r, b_r, W_h, b_h, w_gx, b_gx):
    """Weight arrays shared by all cores (fp32, host layout)."""
    def split(Wf):
        return Wf[:, :D], Wf[:, D:D + H], Wf[:, D + H:]

    Wzx, Wzh, Wzm = split(W_z)
    Wrx, Wrh, Wrm = split(W_r)
    Whx, Whh_, Whm = split(W_h)

    def hid_t(Wh):
        # Wh [H, H] -> Wh.T tiles: [128, MC*H] with tile (k,m) at k*H + m*128
        return (
            Wh.T.reshape(MC, 128, H).transpose(1, 0, 2).reshape(128, MC * H)
        )

    f32 = np.float32
    return {
        "wgx_n": np.ascontiguousarray(-w_gx[:, None], f32),
        "bgx_n": np.ascontiguousarray(-b_gx[:, None], f32),
        "wgh_t": np.ascontiguousarray(W_gh.T, f32),
        "bgh_n": np.ascontiguousarray(-b_gh.reshape(MC, 128).T, f32),
        "wx_t": np.ascontiguousarray(
            np.concatenate([Wzx.T, Wrx.T, Whx.T], axis=1), f32),
        "wm_t": np.ascontiguousarray(
            np.concatenate([Wzm.T, Wrm.T, Whm.T], axis=1), f32),
        "wh_t": np.ascontiguousarray(
            np.concatenate([hid_t(Wzh), hid_t(Wrh), hid_t(Whh_)], axis=1), f32),
        "bias": np.ascontiguousarray(
            np.concatenate(
                [b.reshape(MC, 128).T for b in (b_z, b_r, b_h)], axis=1), f32),
    }


def prep_core(X, X_last_obsv, Mask, Delta, xm_fm, shared, c, Tn):
    sl = slice(c * BL, (c + 1) * BL)
    m = {
        "x": _feature_major(X[sl], Tn),
        "xl": _feature_major(X_last_obsv[sl], Tn),
        "m": _feature_major(Mask[sl], Tn),
        "dt": _feature_major(Delta[sl], Tn),
        "xm": xm_fm,
    }
    m.update(shared)
    return m


def host_finish(h_outs, W_fc, b_fc, bn_gamma, bn_beta):
    """Gather per-core h_last, project to logits, batch-norm over batch."""
    h_last = np.concatenate(
        [o.reshape(128, MC, BL).transpose(2, 1, 0).reshape(BL, H)
         for o in h_outs], axis=0)                      # [B, H]
    logits = h_last.astype(np.float32) @ W_fc.T.astype(np.float32) + b_fc
    mu = logits.mean(axis=0)
    var = logits.var(axis=0)
    out = bn_gamma * (logits - mu) / np.sqrt(var + BN_EPS) + bn_beta
    return out.astype(np.float32)


def run_cores(inputs, Tn=T_FULL, trace=False):
    from concourse.bass_utils import run_bass_kernel_spmd

    inputs = {k: np.asarray(v, dtype=np.float32) for k, v in inputs.items()}
    nc = get_nc(Tn)
    shared = prep_shared(
        inputs["W_gh"], inputs["b_gh"], inputs["W_z"], inputs["b_z"],
        inputs["W_r"], inputs["b_r"], inputs["W_h"], inputs["b_h"],
        inputs["w_gx"], inputs["b_gx"],
    )
    xm_fm = np.ascontiguousarray(
        np.broadcast_to(
            inputs["x_mean"].transpose(2, 1, 0), (D, Tn, BL)
        ).reshape(D, Tn * BL), np.float32)
    in_maps = [
        prep_core(inputs["X"], inputs["X_last_obsv"], inputs["Mask"],
                  inputs["Delta"], xm_fm, shared, c, Tn)
        for c in range(NCORES)
    ]
    res = run_bass_kernel_spmd(
        nc, in_maps, list(range(NCORES)), trace=trace,
    )
    h_outs = [res.results[c]["h_out"] for c in range(NCORES)]
    out = host_finish(h_outs, inputs["W_fc"], inputs["b_fc"],
                      inputs["bn_gamma"], inputs["bn_beta"])
    return out, res


def kernel(**inputs):
    out, _ = run_cores(inputs, Tn=T_FULL, trace=False)
    return out



# revision 14
# speedup vs baseline: 3.1517x; 3.1517x over previous
"""GRU-D Trainium2 kernel (8-core SPMD, data-parallel over batch).

Model (reference): B=512, T=200, D=128, H=512.
Per-core: 64 batch samples, full T recurrence.

Decomposition
-------------
All h-independent terms are precomputed as large parallel matmuls (phase A):
    delta_x = min(1, exp(-(d*w_gx + b_gx)))                  [elementwise]
    xhat    = m*x + (1-m)*(delta_x*xl + (1-delta_x)*xm)      [elementwise]
    delta_h = min(1, exp(-(Wgh @ d + b_gh)))                 [D->H matmul]
    P_g     = Wgx_g @ xhat + Wgm_g @ m + b_g   for g in z,r,h
(split of W_g [H, 2D+H] into x / h / m column blocks; comb = [x, h, m]).

The serial scan (phase B) then only needs, per step:
    g  = delta_h_t * h
    z|r = sigmoid(P_zr_t + W{z,r}h @ g)
    u  = r * g
    ht = tanh(P_h_t + Whh @ u)
    h  = g + z*(ht - g)

Everything on-device is stored feature-major: tensors [H, B_local] live as
SBUF tiles [128, 4*64] with column index = h_chunk*64 + b.  Weights are
stationary [128,128] lhsT tiles, the moving operand is the state (N=64).

Phase A streams per-step records to a DRAM scratch laid out
    rec[t] = [dh (256) | Pz (256) | Pr (256) | Ph (256)]   (cols, per partition)
which phase B consumes with one DMA per step.

Final projection (H->2) + batch norm run on host over the gathered
h_last (trivial FLOPs, needs cross-core batch statistics anyway).
"""

import sys

for _p in ("/opt/trn_rl_repo",):
    if _p not in sys.path:
        sys.path.insert(0, _p)

import numpy as np
from ml_dtypes import bfloat16

import concourse.bacc as bacc
import concourse.tile as tile
from concourse import mybir

AF = mybir.ActivationFunctionType
F32 = mybir.dt.float32
BF16 = mybir.dt.bfloat16

B, T_FULL, D, H = 512, 200, 128, 512
NCORES = 8
BL = B // NCORES          # 64 samples per core
MC = H // 128             # 4 h-chunks
REC = 4 * 256             # per-step record width (dh | Pz | Pr | Ph)
CHUNK = 512               # phase-A tb columns per chunk (= 8 steps worth)
TPC = CHUNK // BL         # timesteps per phase-A chunk (8)
BN_EPS = 1e-5

_nc_cache = {}


def build(T=T_FULL):
    """Build the single-core Bass program (SPMD: same program on all cores)."""
    assert T % TPC == 0
    TB = T * BL
    nchunk = TB // CHUNK

    nc = bacc.Bacc("TRN2", target_bir_lowering=False, debug=False)

    # --- external inputs (feature-major, host-prepared) ---
    def din(name, shape, dt=F32):
        return nc.dram_tensor(name, shape, dt, kind="ExternalInput")

    x_d = din("x", [128, TB], BF16)
    xl_d = din("xl", [128, TB], BF16)
    m_d = din("m", [128, TB], BF16)
    dt_d = din("dt", [128, TB], BF16)
    xm_d = din("xm", [128, TB], BF16)

    wgx_d = din("wgx_n", [128, 1])      # -w_gx
    bgx_d = din("bgx_n", [128, 1])      # -b_gx
    wgh_d = din("wgh_t", [128, H], BF16)   # Wgh.T
    bgh_d = din("bgh_n", [128, MC])     # -b_gh  (col = h chunk)

    wxs_d = din("wx_t", [128, 3 * H], BF16)   # [Wzx.T | Wrx.T | Whx.T]
    wms_d = din("wm_t", [128, 3 * H], BF16)   # [Wzm.T | Wrm.T | Whm.T]
    whh_d = din("wh_t", [128, 3 * MC * H], BF16)  # z|r|h hidden blocks,
    #                                         tile (k,m) at g*4096 + k*512 + m*128
    bia_d = din("bias", [128, 3 * MC])  # b_z | b_r | b_h  (col = g*4 + chunk)

    h_out = nc.dram_tensor("h_out", [128, MC * BL], F32, kind="ExternalOutput")

    rec_s = nc.dram_tensor("rec_s", [128, T * REC], BF16)

    with tile.TileContext(nc) as tc:
        with (
            tc.tile_pool(name="wsb", bufs=1) as wp,
            tc.tile_pool(name="state", bufs=1) as stp,
        ):
            # resident weights
            wgx = wp.tile([128, 1], F32, tag="wgx")
            bgx = wp.tile([128, 1], F32, tag="bgx")
            wgh = wp.tile([128, H], BF16, tag="wgh")
            bgh = wp.tile([128, MC], F32, tag="bgh")
            wxs = wp.tile([128, 3 * H], BF16, tag="wxs")
            wms = wp.tile([128, 3 * H], BF16, tag="wms")
            whh = wp.tile([128, 3 * MC * H], BF16, tag="whh")
            bia = wp.tile([128, 3 * MC], F32, tag="bia")
            for sb_t, dr in [
                (wgx, wgx_d), (bgx, bgx_d), (wgh, wgh_d), (bgh, bgh_d),
                (wxs, wxs_d), (wms, wms_d), (whh, whh_d), (bia, bia_d),
            ]:
                nc.sync.dma_start(sb_t[:], dr[:])

            h = stp.tile([128, MC * BL], F32, tag="h")
            nc.vector.memset(h[:], 0.0)

            # ---------------- phase A ----------------
            with (
                tc.tile_pool(name="pin", bufs=3) as pin,
                tc.tile_pool(name="pw", bufs=2) as pw,
                tc.tile_pool(name="pout", bufs=2) as pout,
                tc.tile_pool(name="psA", bufs=4, space="PSUM") as psA,
            ):
                for ci in range(nchunk):
                    s = ci * CHUNK
                    t0 = ci * TPC
                    xt = pin.tile([128, CHUNK], BF16, tag="x")
                    xlt = pin.tile([128, CHUNK], BF16, tag="xl")
                    mt = pin.tile([128, CHUNK], BF16, tag="m")
                    dtt = pin.tile([128, CHUNK], BF16, tag="d")
                    xmt = pin.tile([128, CHUNK], BF16, tag="xm")
                    nc.sync.dma_start(xt[:], x_d[:, s:s + CHUNK])
                    nc.sync.dma_start(xlt[:], xl_d[:, s:s + CHUNK])
                    nc.sync.dma_start(mt[:], m_d[:, s:s + CHUNK])
                    nc.sync.dma_start(dtt[:], dt_d[:, s:s + CHUNK])
                    nc.sync.dma_start(xmt[:], xm_d[:, s:s + CHUNK])

                    # delta_h: 4 matmuls [128,128]x[128,512] + exp + min
                    dh_sb = pout.tile([128, TPC, 4 * BL], BF16, tag="dh")
                    for mi in range(MC):
                        pdm = psA.tile([128, CHUNK], F32, tag="psA")
                        nc.tensor.matmul(
                            pdm[:], wgh[:, mi * 128:(mi + 1) * 128], dtt[:],
                            start=True, stop=True,
                        )
                        nc.scalar.activation(
                            dh_sb[:, :, mi * BL:(mi + 1) * BL],
                            pdm[:].rearrange("p (t b) -> p t b", b=BL),
                            AF.Exp, bias=bgh[:, mi:mi + 1], scale=-1.0,
                        )
                    dh_flat = dh_sb[:].rearrange("p t b -> p (t b)")
                    nc.vector.tensor_scalar_min(dh_flat, dh_flat, 1.0)
                    rec_v = rec_s[:, t0 * REC:(t0 + TPC) * REC].rearrange(
                        "p (t c) -> p t c", c=REC
                    )
                    nc.sync.dma_start(rec_v[:, :, 0:256], dh_sb[:])

                    # delta_x and xhat (elementwise)
                    dxe = pw.tile([128, CHUNK], F32, tag="dxe")
                    nc.scalar.activation(
                        dxe[:], dtt[:], AF.Exp, bias=bgx[:, 0:1], scale=wgx[:, 0:1]
                    )
                    dx = pw.tile([128, CHUNK], F32, tag="dx")
                    nc.vector.tensor_scalar_min(dx[:], dxe[:], 1.0)
                    t1 = pw.tile([128, CHUNK], F32, tag="t1")
                    nc.vector.tensor_sub(t1[:], xlt[:], xmt[:])
                    t2 = pw.tile([128, CHUNK], F32, tag="t2")
                    nc.vector.tensor_mul(t2[:], dx[:], t1[:])
                    t2b = pw.tile([128, CHUNK], F32, tag="t2b")
                    nc.vector.tensor_add(t2b[:], t2[:], xmt[:])
                    t3 = pw.tile([128, CHUNK], F32, tag="t3")
                    nc.vector.tensor_sub(t3[:], xt[:], t2b[:])
                    t4 = pw.tile([128, CHUNK], F32, tag="t4")
                    nc.vector.tensor_mul(t4[:], mt[:], t3[:])
                    xh = pw.tile([128, CHUNK], BF16, tag="xh")
                    nc.vector.tensor_add(xh[:], t4[:], t2b[:])

                    # P_g = Wgx_g @ xhat + Wgm_g @ m + b_g
                    prec = pout.tile([128, TPC, 3 * 4 * BL], BF16, tag="prec")
                    for gi in range(3):
                        for mi in range(MC):
                            pp = psA.tile([128, CHUNK], F32, tag="psA")
                            wcol = gi * H + mi * 128
                            nc.tensor.matmul(
                                pp[:], wxs[:, wcol:wcol + 128], xh[:],
                                start=True, stop=False,
                            )
                            nc.tensor.matmul(
                                pp[:], wms[:, wcol:wcol + 128], mt[:],
                                start=False, stop=True,
                            )
                            dst = prec[:, :, gi * 256 + mi * BL:
                                       gi * 256 + (mi + 1) * BL]
                            src = pp[:].rearrange("p (t b) -> p t b", b=BL)
                            b_ap = bia[:, gi * MC + mi:gi * MC + mi + 1]
                            if (gi * MC + mi) % 2 == 0:
                                nc.scalar.activation(
                                    dst, src, AF.Identity, bias=b_ap
                                )
                            else:
                                nc.vector.tensor_scalar_add(dst, src, b_ap)
                    nc.sync.dma_start(rec_v[:, :, 256:1024], prec[:])

            # ---------------- phase B (serial scan) ----------------
            with (
                tc.tile_pool(name="prc", bufs=8) as prc,
                tc.tile_pool(name="pgB", bufs=2) as pgB,
                tc.tile_pool(name="pwB", bufs=2) as pwB,
                tc.tile_pool(name="pzr", bufs=2, space="PSUM") as pzr_p,
                tc.tile_pool(name="pht", bufs=2, space="PSUM") as pht_p,
            ):
                W = MC * BL  # 256
                for t in range(T):
                    rec = prc.tile([128, REC], BF16, tag="rec")
                    nc.sync.dma_start(rec[:], rec_s[:, t * REC:(t + 1) * REC])

                    g = pgB.tile([128, W], BF16, tag="g")
                    nc.vector.tensor_mul(g[:], rec[:, 0:W], h[:])

                    pzr = pzr_p.tile([128, 512], F32, tag="pzr")
                    for gi in range(2):
                        for mi in range(MC):
                            o = gi * W + mi * BL
                            for k in range(MC):
                                wcol = gi * MC * H + k * H + mi * 128
                                nc.tensor.matmul(
                                    pzr[:, o:o + BL],
                                    whh[:, wcol:wcol + 128],
                                    g[:, k * BL:(k + 1) * BL],
                                    start=(k == 0), stop=(k == MC - 1),
                                )
                    pre = pwB.tile([128, 512], F32, tag="pre")
                    nc.vector.tensor_add(pre[:], pzr[:], rec[:, 256:768])
                    zr = pwB.tile([128, 512], F32, tag="zr")
                    nc.scalar.activation(zr[:], pre[:], AF.Sigmoid)

                    u = pwB.tile([128, W], BF16, tag="u")
                    nc.vector.tensor_mul(u[:], zr[:, W:2 * W], g[:])

                    pht = pht_p.tile([128, W], F32, tag="pht")
                    for mi in range(MC):
                        o = mi * BL
                        for k in range(MC):
                            wcol = 2 * MC * H + k * H + mi * 128
                            nc.tensor.matmul(
                                pht[:, o:o + BL],
                                whh[:, wcol:wcol + 128],
                                u[:, k * BL:(k + 1) * BL],
                                start=(k == 0), stop=(k == MC - 1),
                            )
                    preh = pwB.tile([128, W], F32, tag="preh")
                    nc.vector.tensor_add(preh[:], pht[:], rec[:, 768:1024])
                    hts = pwB.tile([128, W], F32, tag="hts")
                    nc.scalar.activation(hts[:], preh[:], AF.Tanh)

                    d1 = pwB.tile([128, W], F32, tag="d1")
                    nc.vector.tensor_sub(d1[:], hts[:], g[:])
                    d2 = pwB.tile([128, W], F32, tag="d2")
                    nc.vector.tensor_mul(d2[:], zr[:, 0:W], d1[:])
                    nc.vector.tensor_add(h[:], g[:], d2[:])

            nc.sync.dma_start(h_out[:], h[:])

    nc.compile()
    return nc


def get_nc(T=T_FULL):
    if T not in _nc_cache:
        _nc_cache[T] = build(T)
    return _nc_cache[T]


# ---------------------------------------------------------------- host prep

def _feature_major(a, Tn):
    """[BL, T, D] -> [D, T*BL] with b fastest."""
    return np.ascontiguousarray(
        a.transpose(2, 1, 0), bfloat16).reshape(D, Tn * BL)


def prep_shared(W_gh, b_gh, W_z, b_z, W_r, b_r, W_h, b_h, w_gx, b_gx):
    """Weight arrays shared by all cores (fp32, host layout)."""
    def split(Wf):
        return Wf[:, :D], Wf[:, D:D + H], Wf[:, D + H:]

    Wzx, Wzh, Wzm = split(W_z)
    Wrx, Wrh, Wrm = split(W_r)
    Whx, Whh_, Whm = split(W_h)

    def hid_t(Wh):
        # Wh [H, H] -> Wh.T tiles: [128, MC*H] with tile (k,m) at k*H + m*128
        return (
            Wh.T.reshape(MC, 128, H).transpose(1, 0, 2).reshape(128, MC * H)
        )

    f32 = np.float32
    return {
        "wgx_n": np.ascontiguousarray(-w_gx[:, None], f32),
        "bgx_n": np.ascontiguousarray(-b_gx[:, None], f32),
        "wgh_t": np.ascontiguousarray(W_gh.T, bfloat16),
        "bgh_n": np.ascontiguousarray(-b_gh.reshape(MC, 128).T, f32),
        "wx_t": np.ascontiguousarray(
            np.concatenate([Wzx.T, Wrx.T, Whx.T], axis=1), bfloat16),
        "wm_t": np.ascontiguousarray(
            np.concatenate([Wzm.T, Wrm.T, Whm.T], axis=1), bfloat16),
        "wh_t": np.ascontiguousarray(
            np.concatenate([hid_t(Wzh), hid_t(Wrh), hid_t(Whh_)], axis=1),
            bfloat16),
        "bias": np.ascontiguousarray(
            np.concatenate(
                [b.reshape(MC, 128).T for b in (b_z, b_r, b_h)], axis=1), f32),
    }


def prep_core(X, X_last_obsv, Mask, Delta, xm_fm, shared, c, Tn):
    sl = slice(c * BL, (c + 1) * BL)
    m = {
        "x": _feature_major(X[sl], Tn),
        "xl": _feature_major(X_last_obsv[sl], Tn),
        "m": _feature_major(Mask[sl], Tn),
        "dt": _feature_major(Delta[sl], Tn),
        "xm": xm_fm,
    }
    m.update(shared)
    return m


def host_finish(h_outs, W_fc, b_fc, bn_gamma, bn_beta):
    """Gather per-core h_last, project to logits, batch-norm over batch."""
    h_last = np.concatenate(
        [o.reshape(128, MC, BL).transpose(2, 1, 0).reshape(BL, H)
         for o in h_outs], axis=0)                      # [B, H]
    logits = h_last.astype(np.float32) @ W_fc.T.astype(np.float32) + b_fc
    mu = logits.mean(axis=0)
    var = logits.var(axis=0)
    out = bn_gamma * (logits - mu) / np.sqrt(var + BN_EPS) + bn_beta
    return out.astype(np.float32)


def run_cores(inputs, Tn=T_FULL, trace=False):
    from concourse.bass_utils import run_bass_kernel_spmd

    inputs = {k: np.asarray(v, dtype=np.float32) for k, v in inputs.items()}
    nc = get_nc(Tn)
    shared = prep_shared(
        inputs["W_gh"], inputs["b_gh"], inputs["W_z"], inputs["b_z"],
        inputs["W_r"], inputs["b_r"], inputs["W_h"], inputs["b_h"],
        inputs["w_gx"], inputs["b_gx"],
    )
    xm_fm = np.ascontiguousarray(
        np.broadcast_to(
            inputs["x_mean"].transpose(2, 1, 0), (D, Tn, BL)
        ), bfloat16).reshape(D, Tn * BL)
    in_maps = [
        prep_core(inputs["X"], inputs["X_last_obsv"], inputs["Mask"],
                  inputs["Delta"], xm_fm, shared, c, Tn)
        for c in range(NCORES)
    ]
    res = run_bass_kernel_spmd(
        nc, in_maps, list(range(NCORES)), trace=trace,
    )
    h_outs = [res.results[c]["h_out"] for c in range(NCORES)]
    out = host_finish(h_outs, inputs["W_fc"], inputs["b_fc"],
                      inputs["bn_gamma"], inputs["bn_beta"])
    return out, res


def kernel(**inputs):
    out, _ = run_cores(inputs, Tn=T_FULL, trace=False)
    return out

